# revision 1
# baseline (speedup 1.0000x reference)
"""AttnBlock on 8 Trainium2 NeuronCores via Bass/Tile.

Reference computation (shapes hardcoded): x (4, 256, 64, 64) f32,
GroupNorm(32 groups) -> q/k/v 1x1 conv -> HWxHW attention (with the
reference's raw-view reshape (C,N)->(N,C) for q and v) -> proj -> x + p.

Sharding: 8 cores = 4 batch elements x 2 query-halves, mesh (pair=4,
half=2). Core (b, j) handles batch b and attention rows n in
[j*2048, (j+1)*2048). The raw view means q_att rows [j*2048,(j+1)*2048)
depend only on wq rows [j*128,(j+1)*128), so each core computes: full
GN(x[b]), full k/v, its half of qT, its half of the attention, and p
columns [j*2048,(j+1)*2048). No collectives. The residual x + p is added
on the host in f32 (p is ~1e-5 scale, so a bf16 p loses nothing).

Key layout identity (N=4096=HW, C=256): q_att[n,c] = q[n//16, 256*(n%16)+c],
so  qT[c, 16a+r] = (h[:, 256r:256r+256].T @ wq_half.T)[c, a]
which lets us build q_att.T (c on partitions) directly with matmuls.

Attention is computed transposed: ST[j,i] = sum_c k[c,j]*qT[c,i], then
E = exp(ST/16 - 4) (scores are ~N(0,1): no max subtraction needed, and
the -4 centers E in fp8 range and cancels in the normalization),
h_attT[c,i] = sum_j v_att[j,c]*E[j,i] accumulated in PSUM over j-block
pairs. k/qT/E/v_att are fp8(e4m3) with K=256 packed [128,2,.] for
DoubleRow matmuls (half the matmul count; errors are damped by the 1e-5
wp scale). The softmax denominator accumulates via M=1 ones-matmuls,
is partition-broadcast with a K=1 ones-matmul, inverted with the fast
approximate reciprocal, and applied after the wp projection (division
by l commutes through the channel-mixing matmul). The accumulation
matmuls for a j-block pair are software-pipelined one pair behind the
score matmuls so the tensor engine never waits on exp.

Host I/O is minimized for the axon tunnel: x ships bf16 once per batch
pair, per-half q weights ship per half, everything else is packed into
one replicated f32 array; outputs are bf16 p-halves.
"""

import numpy as np

B, C, H, W = 4, 256, 64, 64
N = H * W            # 4096 pixels
HALF = N // 2        # 2048 attention rows per core
GROUPS = 32
GSIZE = C // GROUPS  # 8 channels per group
EPS = 1e-5
NCORES = 8
P = 128              # partitions
NB = N // P          # 32 j-blocks of 128
SC = 4               # i super-chunks per core
CHUNK = 512          # i columns per chunk (one PSUM bank)

# wpack column layout (f32, C rows)
WK0, WV0, WP0 = 0, C, 2 * C
BK0, BV0, BP0, GW0, GB0 = 3 * C, 3 * C + 1, 3 * C + 2, 3 * C + 3, 3 * C + 4
IND0 = 3 * C + 5            # [C, 32] group one-hot
INDT0 = IND0 + GROUPS       # [32, 2*128] transposed one-hot (rows 0..31)
WCOLS = INDT0 + C

_cache = {}


def _build_nc():
    import concourse.tile as tile
    from concourse import bacc, mybir

    f32 = mybir.dt.float32
    bf16 = mybir.dt.bfloat16
    f8 = mybir.dt.float8e4
    AF = mybir.ActivationFunctionType
    OP = mybir.AluOpType

    nc = bacc.Bacc("TRN2", target_bir_lowering=False, debug=False,
                   num_devices=NCORES)

    x_ap = nc.dram_tensor("x", [C, N], bf16, kind="ExternalInput").ap()
    qpack_ap = nc.dram_tensor("qpack", [C + 1, P], f32,
                              kind="ExternalInput").ap()
    wpack_ap = nc.dram_tensor("wpack", [C, WCOLS], f32,
                              kind="ExternalInput").ap()
    out_ap = nc.dram_tensor("out", [C, HALF], bf16, kind="ExternalOutput").ap()

    with tile.TileContext(nc) as tc:
        with (
            tc.tile_pool(name="persist", bufs=1) as persist,
            tc.tile_pool(name="wtmp", bufs=2) as wtmp,
            tc.tile_pool(name="small", bufs=4) as small,
            tc.tile_pool(name="epool", bufs=6) as epool,
            tc.tile_pool(name="htpool", bufs=8) as htpool,
            tc.tile_pool(name="opool", bufs=4) as opool,
            tc.tile_pool(name="mm", bufs=2, space="PSUM") as mm,
            tc.tile_pool(name="hacc", bufs=2, space="PSUM") as hacc,
            tc.tile_pool(name="lacc", bufs=2, space="PSUM") as lacc,
        ):
            # ---------- load x and weights ----------
            # x arrives in 1024-col chunks so bn_stats can start while the
            # rest of the transfer is still in flight
            x_sb = []
            for cb in range(2):
                t = persist.tile([P, N], bf16, tag=f"x{cb}", name=f"x{cb}")
                for q in range(4):
                    csl = slice(q * 1024, (q + 1) * 1024)
                    eng = nc.sync if cb == 0 else nc.scalar
                    eng.dma_start(t[:, csl],
                                  x_ap[cb * P:(cb + 1) * P, csl])
                x_sb.append(t)

            _conv_n = [0]

            def load_conv(ap, rows, cols, tag, dt, dst=None):
                """DMA f32 DRAM -> SBUF (off the x DMA queue), convert;
                converts alternate DVE/ACT so neither delays bn_stats."""
                tmp = wtmp.tile([rows, cols], f32, tag="wload", name="wload")
                nc.gpsimd.dma_start(tmp[:], ap)
                if dst is None:
                    dst = persist.tile([rows, cols], dt, tag=tag, name=tag)
                _conv_n[0] += 1
                if _conv_n[0] % 2:
                    nc.vector.tensor_copy(dst[:], tmp[:])
                else:
                    nc.scalar.activation(dst[:], tmp[:], AF.Identity)
                return dst

            # keep the PE busy through the x-DMA/GN phase: dummy matmuls
            # consuming each x chunk as it arrives hold the HAM clock-gate
            # at full rate so the convs and attention start warm
            warm_w = persist.tile([P, P], bf16, tag="warmw", name="warmw")
            nc.gpsimd.memset(warm_w[:], 1.0)
            for rep in range(2):
                for cb in range(2):
                    for q in range(4):
                        junk = mm.tile([P, 2, CHUNK], f32, tag="mm",
                                       name="mm")
                        nc.tensor.matmul(
                            junk[:, 0, :], warm_w[:],
                            x_sb[cb][:, q * 1024 + rep * CHUNK:
                                      q * 1024 + (rep + 1) * CHUNK],
                            start=True, stop=True)

            def load_bf16(ap, rows, cols, tag):
                return load_conv(ap, rows, cols, tag, bf16)

            def rows(cb):
                return slice(cb * P, (cb + 1) * P)

            wqt_bf = [load_bf16(qpack_ap[rows(cb), :], P, P, f"wqt{cb}")
                      for cb in range(2)]
            wkt_bf = [load_bf16(wpack_ap[rows(cb), WK0:WK0 + C], P, C,
                                f"wkt{cb}") for cb in range(2)]
            wvt_bf = [load_bf16(wpack_ap[rows(cb), WV0:WV0 + C], P, C,
                                f"wvt{cb}") for cb in range(2)]
            bq_bf = load_bf16(qpack_ap[C:C + 1, :], 1, P, "bqrow")
            wpt_bf = [load_bf16(wpack_ap[rows(cb), WP0:WP0 + C], P, C,
                                f"wpt{cb}") for cb in range(2)]

            def load_f32(ap, nrows, cols, tag):
                t = persist.tile([nrows, cols], f32, tag=tag, name=tag)
                nc.gpsimd.dma_start(t[:], ap)
                return t

            bk_sb = [load_f32(wpack_ap[rows(cb), BK0:BK0 + 1], P, 1, f"bk{cb}")
                     for cb in range(2)]
            bv_sb = [load_f32(wpack_ap[rows(cb), BV0:BV0 + 1], P, 1, f"bv{cb}")
                     for cb in range(2)]
            bp_sb = [load_f32(wpack_ap[rows(cb), BP0:BP0 + 1], P, 1, f"bp{cb}")
                     for cb in range(2)]
            gnw_sb = [load_f32(wpack_ap[rows(cb), GW0:GW0 + 1], P, 1,
                               f"gnw{cb}") for cb in range(2)]
            gnb_sb = [load_f32(wpack_ap[rows(cb), GB0:GB0 + 1], P, 1,
                               f"gnb{cb}") for cb in range(2)]
            ind_sb = [load_f32(wpack_ap[rows(cb), IND0:IND0 + GROUPS], P,
                               GROUPS, f"ind{cb}") for cb in range(2)]
            indt_sb = [load_f32(
                wpack_ap[0:GROUPS, INDT0 + cb * P:INDT0 + (cb + 1) * P],
                GROUPS, P, f"indt{cb}") for cb in range(2)]

            ones_row_bf = persist.tile([1, P], bf16, tag="ones_bf",
                                       name="ones_bf")
            nc.vector.memset(ones_row_bf[:], 1.0)
            ones_row_f = persist.tile([1, P], f32, tag="ones_f", name="ones_f")
            nc.vector.memset(ones_row_f[:], 1.0)
            # [P, 2, 16] so the DoubleRow pair-dim step (16) satisfies the
            # fp8 dual-row LDWEIGHTS step%16==0 restriction; only col 0 used
            ones_pair_f8 = persist.tile([P, 2, 16], f8, tag="ones_pair",
                                        name="ones_pair")
            nc.vector.memset(ones_pair_f8[:], 1.0)
            eps_sb = persist.tile([GROUPS, 1], f32, tag="eps", name="eps")
            nc.vector.memset(eps_sb[:], EPS)
            neg4_sb = persist.tile([P, 1], f32, tag="neg4", name="neg4")
            nc.vector.memset(neg4_sb[:], -4.0)

            # ---------- GroupNorm ----------
            # per-channel mean / E[x^2] on 128-channel tiles
            m1m2 = []
            for cb in range(2):
                xv = x_sb[cb].rearrange("p (s q) -> p s q", q=512)
                stats = small.tile([P, 8, 6], f32, tag="bnstats",
                                   name="bnstats")
                for s in range(8):
                    nc.vector.bn_stats(stats[:, s, :], xv[:, s, :])
                mv = small.tile([P, 2], f32, tag="bnmv", name="bnmv")
                nc.vector.bn_aggr(mv[:], stats[:])
                mm12 = small.tile([P, 2], f32, tag="m1m2", name="m1m2")
                nc.vector.tensor_copy(mm12[:, 0:1], mv[:, 0:1])
                sq = small.tile([P, 1], f32, tag="gnsq", name="gnsq")
                nc.vector.tensor_mul(sq[:], mv[:, 0:1], mv[:, 0:1])
                nc.vector.tensor_add(mm12[:, 1:2], mv[:, 1:2], sq[:])
                m1m2.append(mm12)

            # group sums: [32, 2] = sum over channels in group
            ps_g = mm.tile([GROUPS, 2], f32, tag="mm", name="mm")
            nc.tensor.matmul(ps_g[:], ind_sb[0][:], m1m2[0][:],
                             start=True, stop=False)
            nc.tensor.matmul(ps_g[:], ind_sb[1][:], m1m2[1][:],
                             start=False, stop=True)

            gstats = small.tile([GROUPS, 2], f32, tag="gstats", name="gstats")
            nc.vector.tensor_scalar_mul(gstats[:, 0:1], ps_g[:, 0:1],
                                        1.0 / GSIZE)
            ex2 = small.tile([GROUPS, 1], f32, tag="gex2", name="gex2")
            nc.vector.tensor_scalar_mul(ex2[:], ps_g[:, 1:2], 1.0 / GSIZE)
            musq = small.tile([GROUPS, 1], f32, tag="gmusq", name="gmusq")
            nc.vector.tensor_mul(musq[:], gstats[:, 0:1], gstats[:, 0:1])
            gvar = small.tile([GROUPS, 1], f32, tag="gvar", name="gvar")
            nc.vector.tensor_sub(gvar[:], ex2[:], musq[:])
            gsd = small.tile([GROUPS, 1], f32, tag="gsd", name="gsd")
            nc.scalar.activation(gsd[:], gvar[:], AF.Sqrt, bias=eps_sb[:])
            nc.vector.reciprocal(gstats[:, 1:2], gsd[:])

            # scatter group stats back to channels, apply GN
            h_bf = []
            for cb in range(2):
                ps_c = mm.tile([P, 2], f32, tag="mm", name="mm")
                nc.tensor.matmul(ps_c[:], indt_sb[cb][:], gstats[:],
                                 start=True, stop=True)
                scale_c = small.tile([P, 1], f32, tag="scalec", name="scalec")
                nc.vector.tensor_mul(scale_c[:], ps_c[:, 1:2], gnw_sb[cb][:])
                mus = small.tile([P, 1], f32, tag="mus", name="mus")
                nc.vector.tensor_mul(mus[:], ps_c[:, 0:1], scale_c[:])
                bias_c = small.tile([P, 1], f32, tag="biasc", name="biasc")
                nc.vector.tensor_sub(bias_c[:], gnb_sb[cb][:], mus[:])
                hb = persist.tile([P, N], bf16, tag=f"h{cb}", name=f"h{cb}")
                if cb == 0:
                    nc.vector.tensor_scalar(hb[:], x_sb[cb][:], scale_c[:],
                                            bias_c[:], op0=OP.mult,
                                            op1=OP.add)
                else:
                    nc.scalar.activation(hb[:], x_sb[cb][:], AF.Identity,
                                         bias=bias_c[:], scale=scale_c[:])
                h_bf.append(hb)

            DR = mybir.MatmulPerfMode.DoubleRow
            # ---------- k, v (full, fp8), then v_att expansion ----------
            # k is stored [c', blk, j] (K=256 packed for DoubleRow); v as
            # two plain [128, N] fp8 tiles that feed the v_att expansion.
            k_f8 = persist.tile([P, 2, N], f8, tag="kf8", name="kf8")
            v_f8 = [persist.tile([P, N], f8, tag=f"vf8{cb}", name=f"vf8{cb}")
                    for cb in range(2)]

            def conv_full(wt_bf, b_sb, dst, copy_eng):
                for cob in range(2):
                    for chp in range(4):
                        ps = mm.tile([P, 2, CHUNK], f32, tag="mm", name="mm")
                        for s in range(2):
                            ch = 2 * chp + s
                            sl = slice(ch * CHUNK, (ch + 1) * CHUNK)
                            nc.tensor.matmul(
                                ps[:, s, :],
                                wt_bf[0][:, cob * P:(cob + 1) * P],
                                h_bf[0][:, sl], start=True, stop=False)
                            nc.tensor.matmul(
                                ps[:, s, :],
                                wt_bf[1][:, cob * P:(cob + 1) * P],
                                h_bf[1][:, sl], start=False, stop=True)
                        for s in range(2):
                            ch = 2 * chp + s
                            sl = slice(ch * CHUNK, (ch + 1) * CHUNK)
                            if copy_eng == "act":
                                nc.scalar.activation(dst(cob, sl),
                                                     ps[:, s, :],
                                                     AF.Identity,
                                                     bias=b_sb[cob][:])
                            else:
                                nc.vector.tensor_scalar_add(dst(cob, sl),
                                                            ps[:, s, :],
                                                            b_sb[cob][:])

            conv_full(wkt_bf, bk_sb, lambda cob, sl: k_f8[:, cob, sl], "dve")
            conv_full(wvt_bf, bv_sb, lambda cob, sl: v_f8[cob][:, sl], "act")

            # v_att[j, c] = v[j//16, 256*(j%16)+c]; stored [j', pair, jlo, c]
            # so a [128, 2, 128] DoubleRow stationary covers two j-blocks.
            v_att = persist.tile([P, NB // 2, 2, C], f8, tag="vatt",
                                 name="vatt")
            for jb in range(NB):
                cb = jb // 16
                p0 = (jb % 16) * 8
                src = v_f8[cb][p0:p0 + 8, :].rearrange("p (r c) -> p r c", c=C)
                nc.sync.dma_start(v_att[:, jb // 2, jb % 2, :], src)

            # ---------- qT: q_att.T for this core's half (fp8, packed) ----
            # emitted after the v_att expansion so those DMAs overlap the
            # qT matmuls
            qT = persist.tile([P, 2, HALF], f8, tag="qT", name="qT")
            qTv = qT.rearrange("p b (a r) -> p b a r", r=16)
            for r in range(16):
                for cb in range(2):
                    ps = mm.tile([P, 2, CHUNK], f32, tag="mm", name="mm")
                    sl = slice(256 * r + cb * P, 256 * r + (cb + 1) * P)
                    nc.tensor.matmul(ps[:, 0, 0:P], h_bf[0][:, sl],
                                     wqt_bf[0][:], start=True, stop=False)
                    nc.tensor.matmul(ps[:, 0, 0:P], h_bf[1][:, sl],
                                     wqt_bf[1][:], start=False, stop=False)
                    nc.tensor.matmul(ps[:, 0, 0:P], ones_row_bf[:], bq_bf[:],
                                     start=False, stop=True)
                    if cb == 0:
                        nc.vector.tensor_copy(qTv[:, cb, :, r], ps[:, 0, 0:P])
                    else:
                        nc.scalar.activation(qTv[:, cb, :, r], ps[:, 0, 0:P],
                                             AF.Identity)

            # ---------- attention + projection ----------
            DR = mybir.MatmulPerfMode.DoubleRow

            def make_stage5(isl, hps, lp):
                def stage5():
                    # broadcast l to 128 partitions via a K=1 matmul, then
                    # a fast approximate reciprocal
                    l_sb = small.tile([1, CHUNK], f32, tag="lsb", name="lsb")
                    nc.vector.tensor_copy(l_sb[:], lp[:])
                    hT = [htpool.tile([P, CHUNK], bf16, tag="hT", name="hT")
                          for _ in range(2)]
                    nc.vector.tensor_copy(hT[0][:], hps[0][:])
                    nc.scalar.activation(hT[1][:], hps[1][:], AF.Identity)
                    l_ps = mm.tile([P, 2, CHUNK], f32, tag="mm", name="mm")
                    nc.tensor.matmul(l_ps[:, 0, :], ones_row_f[:], l_sb[:],
                                     start=True, stop=True)
                    rl_bc = opool.tile([P, CHUNK], f32, tag="rlbc",
                                       name="rlbc")
                    nc.vector.reciprocal_approx_fast(rl_bc[:], l_ps[:, 0, :])
                    for cob in range(2):
                        pp = mm.tile([P, 2, CHUNK], f32, tag="mm", name="mm")
                        nc.tensor.matmul(pp[:, 0, :],
                                         wpt_bf[0][:, cob * P:(cob + 1) * P],
                                         hT[0][:], start=True, stop=False)
                        nc.tensor.matmul(pp[:, 0, :],
                                         wpt_bf[1][:, cob * P:(cob + 1) * P],
                                         hT[1][:], start=False, stop=True)
                        t1 = opool.tile([P, CHUNK], f32, tag="t1", name="t1")
                        nc.vector.tensor_mul(t1[:], pp[:, 0, :], rl_bc[:])
                        o_t = opool.tile([P, CHUNK], bf16, tag="ot",
                                         name="ot")
                        nc.vector.tensor_scalar_add(o_t[:], t1[:],
                                                    bp_sb[cob][:])
                        nc.sync.dma_start(
                            out_ap[cob * P:(cob + 1) * P, isl], o_t[:])
                return stage5

            # stage 5 of super-chunk n is deferred into super-chunk n+1's
            # loop (after two score iterations are in flight) so its serial
            # copy/reciprocal chain never stalls the tensor engine; the
            # h/l PSUM tiles are allocated only after the previous chunk's
            # stage 5 has been emitted (i.e. after their release points).
            pending5 = None
            for sc in range(SC):
                isl = slice(sc * CHUNK, (sc + 1) * CHUNK)
                hps = None
                lp = None

                def accums(pair, e_pair):
                    """h_attT and softmax-denominator accumulation for a
                    pair of j-blocks (DoubleRow, K=256)."""
                    st, sp = (pair == 0), (pair == NB // 2 - 1)
                    nc.tensor.matmul(hps[0][:], v_att[:, pair, :, 0:P],
                                     e_pair[:], start=st, stop=sp,
                                     perf_mode=DR)
                    nc.tensor.matmul(hps[1][:], v_att[:, pair, :, P:C],
                                     e_pair[:], start=st, stop=sp,
                                     perf_mode=DR)
                    nc.tensor.matmul(lp[:], ones_pair_f8[:, :, 0:1],
                                     e_pair[:], start=st, stop=sp,
                                     perf_mode=DR)

                prev = None
                for m in range(NB // 2):
                    ps_s = mm.tile([P, 2, CHUNK], f32, tag="mm", name="mm")
                    for s in range(2):
                        jb = 2 * m + s
                        jsl = slice(jb * P, (jb + 1) * P)
                        nc.tensor.matmul(ps_s[:, s, :], k_f8[:, :, jsl],
                                         qT[:, :, isl], start=True,
                                         stop=True, perf_mode=DR)
                    e_pair = epool.tile([P, 2, CHUNK], f8, tag="e", name="e")
                    # e^{s/16 - 4}: the -4 keeps E in fp8's finite range and
                    # cancels exactly in the softmax normalization
                    nc.scalar.activation(e_pair[:], ps_s[:], AF.Exp,
                                         scale=float(C) ** -0.5,
                                         bias=neg4_sb[:])
                    if m == 1 and pending5 is not None:
                        pending5()
                        pending5 = None
                    if prev is not None:
                        if hps is None:
                            hps = [hacc.tile([P, CHUNK], f32, tag="hacc",
                                             name="hacc") for _ in range(2)]
                            lp = lacc.tile([1, CHUNK], f32, tag="lacc",
                                           name="lacc")
                        accums(*prev)
                    prev = (m, e_pair)
                accums(*prev)
                pending5 = make_stage5(isl, hps, lp)
            pending5()

    nc.compile()
    return nc


def _get_exec():
    if "fn" in _cache:
        return _cache["fn"], _cache["zfn"], _cache["in_names"]

    import jax
    import jax.numpy as jnp
    import ml_dtypes
    from jax.experimental.shard_map import shard_map
    from jax.sharding import Mesh, NamedSharding, PartitionSpec as PS

    from concourse import bass2jax, mybir

    try:
        jax.config.update("jax_compilation_cache_dir", "/tmp/jax_cc_cache")
        jax.config.update("jax_persistent_cache_min_compile_time_secs", 0.0)
    except Exception:
        pass

    nc = _build_nc()
    _cache["nc"] = nc
    bass2jax.install_neuronx_cc_hook()

    partition_name = (nc.partition_id_tensor.name
                      if nc.partition_id_tensor else None)
    in_names, out_names, out_avals = [], [], []
    for alloc in nc.m.functions[0].allocations:
        if not isinstance(alloc, mybir.MemoryLocationSet):
            continue
        name = alloc.memorylocations[0].name
        if alloc.kind == "ExternalInput":
            if name != partition_name:
                in_names.append(name)
        elif alloc.kind == "ExternalOutput":
            out_avals.append(jax.core.ShapedArray(
                tuple(alloc.tensor_shape), mybir.dt.np(alloc.dtype)))
            out_names.append(name)
    n_params = len(in_names)
    all_in_names = in_names + out_names
    if partition_name:
        all_in_names = all_in_names + [partition_name]

    def _body(*args):
        operands = list(args)
        if partition_name:
            operands.append(bass2jax.partition_id_tensor())
        outs = bass2jax._bass_exec_p.bind(
            *operands, out_avals=tuple(out_avals),
            in_names=tuple(all_in_names), out_names=tuple(out_names),
            lowering_input_output_aliases=(), sim_require_finite=True,
            sim_require_nnan=True, nc=nc)
        return tuple(outs)

    devices = np.asarray(jax.devices()[:NCORES]).reshape(B, 2)
    mesh = Mesh(devices, ("pair", "half"))
    spec_by_name = {"x": PS("pair"), "qpack": PS("half"), "wpack": PS()}
    in_specs = tuple(spec_by_name[n] for n in in_names) + (PS(("pair", "half")),)
    out_specs = (PS(("pair", "half")),)

    fn = jax.jit(
        shard_map(_body, mesh=mesh, in_specs=in_specs,
                  out_specs=out_specs, check_rep=False),
        donate_argnums=(n_params,), keep_unused=True)

    zsharding = NamedSharding(mesh, PS(("pair", "half")))
    zfn = jax.jit(
        lambda: jnp.zeros((NCORES * C, HALF), ml_dtypes.bfloat16),
        out_shardings=zsharding)

    _cache["fn"] = fn
    _cache["zfn"] = zfn
    _cache["in_names"] = in_names
    return fn, zfn, in_names


def _pack_inputs(x, gn_w, gn_b, wq, bq, wk, bk, wv, bv, wp, bp):
    import ml_dtypes
    f = np.float32
    asrt = lambda a: np.asarray(a, f)
    x = np.asarray(x, f).reshape(B * C, N)
    x_bf = x.astype(ml_dtypes.bfloat16)

    wq, wk, wv, wp = asrt(wq), asrt(wk), asrt(wv), asrt(wp)
    bq, bk, bv, bp = asrt(bq), asrt(bk), asrt(bv), asrt(bp)
    gn_w, gn_b = asrt(gn_w), asrt(gn_b)

    qpack = np.empty((2 * (C + 1), P), f)
    wqT = wq.T
    for j in range(2):
        qpack[j * (C + 1):j * (C + 1) + C] = wqT[:, j * P:(j + 1) * P]
        qpack[j * (C + 1) + C] = bq[j * P:(j + 1) * P]

    wpack = np.zeros((C, WCOLS), f)
    wpack[:, WK0:WK0 + C] = wk.T
    wpack[:, WV0:WV0 + C] = wv.T
    wpack[:, WP0:WP0 + C] = wp.T
    wpack[:, BK0] = bk
    wpack[:, BV0] = bv
    wpack[:, BP0] = bp
    wpack[:, GW0] = gn_w
    wpack[:, GB0] = gn_b
    ind = np.zeros((C, GROUPS), f)
    ind[np.arange(C), np.arange(C) // GSIZE] = 1.0
    wpack[:, IND0:IND0 + GROUPS] = ind
    wpack[0:GROUPS, INDT0:INDT0 + C] = ind.T
    return x, x_bf, qpack, wpack


def kernel(x, gn_w, gn_b, wq, bq, wk, bk, wv, bv, wp, bp):
    fn, zfn, in_names = _get_exec()
    x_f32, x_bf, qpack, wpack = _pack_inputs(
        x, gn_w, gn_b, wq, bq, wk, bk, wv, bv, wp, bp)
    arrs = {"x": x_bf, "qpack": qpack, "wpack": wpack}
    (p_out,) = fn(*(arrs[n] for n in in_names), zfn())
    # p_out: (8*C, HALF) bf16, blocks ordered core = 2b + j
    p = np.asarray(p_out).astype(np.float32).reshape(B, 2, C, HALF)
    out = np.empty((B, C, N), np.float32)
    for j in range(2):
        out[:, :, j * HALF:(j + 1) * HALF] = p[:, j]
    out += x_f32.reshape(B, C, N)
    return out.reshape(B, C, H, W)



# revision 2
# speedup vs baseline: 3492.2073x; 3492.2073x over previous
"""AttnBlock on 8 Trainium2 NeuronCores via Bass/Tile.

Reference computation (shapes hardcoded): x (4, 256, 64, 64) f32,
GroupNorm(32 groups) -> q/k/v 1x1 conv -> HWxHW attention (with the
reference's raw-view reshape (C,N)->(N,C) for q and v) -> proj -> x + p.

Sharding: 8 cores = 4 batch elements x 2 query-halves, mesh (pair=4,
half=2). Core (b, j) handles batch b and attention rows n in
[j*2048, (j+1)*2048). The raw view means q_att rows [j*2048,(j+1)*2048)
depend only on wq rows [j*128,(j+1)*128), so each core computes: full
k/v, its half of qT, its half of the attention, and p columns
[j*2048,(j+1)*2048). No collectives. The residual x + p is added on the
host in f32 (p is ~1e-5 scale, so a bf16 p loses nothing).

GroupNorm is FOLDED into the q/k/v conv weights: h = scale_c*x + bias_c
per channel, so W@h = (W*scale)@x + (W@bias_c). The per-channel scale is
multiplied into the weight converts (f32->bf16, same op count) and the
bias terms become tiny matmuls, so h is never materialized and the convs
consume x directly. The q-side bias is per out-column of the qT tiles
and is added during the PSUM->SBUF copies via scalar_tensor_tensor with
a partition-broadcast bq row.

Key layout identity (N=4096=HW, C=256): q_att[n,c] = q[n//16, 256*(n%16)+c],
so  qT[c, 16a+r] = (x[:, 256r:256r+256].T @ wq_half'.T)[c, a]
which lets us build q_att.T (c on partitions) directly with matmuls.

Attention is computed transposed: ST[j,i] = sum_c k[c,j]*qT[c,i], then
E = exp(ST/16 - 4) (scores are ~N(0,1): no max subtraction needed, and
the -4 centers E in fp8 range and cancels in the normalization),
h_attT[c,i] = sum_j v_att[j,c]*E[j,i] accumulated in PSUM over j-block
pairs. k/qT/E/v_att are fp8(e4m3) with K=256 packed [128,2,.] for
DoubleRow matmuls. Scores for TWO j-block pairs land in one [128,4,512]
PSUM quad and one ACT exp covers all four blocks ((N+352)/1.2 ns per
ACTIVATE makes batching ~15% cheaper, and the fewer sync points keep the
PE from ping-ponging with ACT). The softmax denominator accumulates via
M=1 ones-matmuls, is inverted on [1,512] and partition-broadcast on
GpSimd (off the PE), and is applied after the wp projection. The
accumulation matmuls for a group run one group behind the score matmuls
so the tensor engine never waits on exp.

Host I/O: x ships bf16 once per batch pair, per-half q weights ship per
half, everything else is packed into one replicated f32 array; outputs
are bf16 p-halves.
"""

import numpy as np

B, C, H, W = 4, 256, 64, 64
N = H * W            # 4096 pixels
HALF = N // 2        # 2048 attention rows per core
GROUPS = 32
GSIZE = C // GROUPS  # 8 channels per group
EPS = 1e-5
NCORES = 8
P = 128              # partitions
NB = N // P          # 32 j-blocks of 128
SC = 4               # i super-chunks per core
CHUNK = 512          # i columns per chunk (one PSUM bank)
NG = NB // 4         # 8 score groups (of 2 j-block pairs) per chunk

# wpack column layout (f32, C rows)
WK0, WV0, WP0 = 0, C, 2 * C
BK0, BV0, BP0, GW0, GB0 = 3 * C, 3 * C + 1, 3 * C + 2, 3 * C + 3, 3 * C + 4
IND0 = 3 * C + 5            # [C, 32] group one-hot
INDT0 = IND0 + GROUPS       # [32, 2*128] transposed one-hot (rows 0..31)
WCOLS = INDT0 + C

_cache = {}


def _build_nc():
    import concourse.tile as tile
    from concourse import bacc, mybir

    f32 = mybir.dt.float32
    bf16 = mybir.dt.bfloat16
    f8 = mybir.dt.float8e4
    AF = mybir.ActivationFunctionType
    OP = mybir.AluOpType

    nc = bacc.Bacc("TRN2", target_bir_lowering=False, debug=False,
                   num_devices=NCORES)

    x_ap = nc.dram_tensor("x", [C, N], bf16, kind="ExternalInput").ap()
    qpack_ap = nc.dram_tensor("qpack", [C + 1, P], f32,
                              kind="ExternalInput").ap()
    wpack_ap = nc.dram_tensor("wpack", [C, WCOLS], f32,
                              kind="ExternalInput").ap()
    out_ap = nc.dram_tensor("out", [C, HALF], bf16, kind="ExternalOutput").ap()

    with tile.TileContext(nc) as tc:
        with (
            tc.tile_pool(name="persist", bufs=1) as persist,
            tc.tile_pool(name="wtmp", bufs=2) as wtmp,
            tc.tile_pool(name="small", bufs=4) as small,
            tc.tile_pool(name="epool", bufs=3) as epool,
            tc.tile_pool(name="htpool", bufs=4) as htpool,
            tc.tile_pool(name="opool", bufs=4) as opool,
        ):
            # ---------- load x (first on its queues) and f32 weights ------
            x_sb = []
            for cb in range(2):
                t = persist.tile([P, N], bf16, tag=f"x{cb}", name=f"x{cb}")
                for q in range(4):
                    csl = slice(q * 1024, (q + 1) * 1024)
                    eng = nc.sync if cb == 0 else nc.scalar
                    eng.dma_start(t[:, csl],
                                  x_ap[cb * P:(cb + 1) * P, csl])
                x_sb.append(t)

            def rows(cb):
                return slice(cb * P, (cb + 1) * P)

            # raw f32 weights; the gpsimd queue runs behind the x DMAs
            def load_f32(ap, nrows, cols, tag):
                t = persist.tile([nrows, cols], f32, tag=tag, name=tag)
                nc.gpsimd.dma_start(t[:], ap)
                return t

            wq_f = [load_f32(qpack_ap[rows(cb), :], P, P, f"wqf{cb}")
                    for cb in range(2)]
            bq_row_f = load_f32(qpack_ap[C:C + 1, :], 1, P, "bqrow")
            wk_f = [load_f32(wpack_ap[rows(cb), WK0:WK0 + C], P, C,
                             f"wkf{cb}") for cb in range(2)]
            wv_f = [load_f32(wpack_ap[rows(cb), WV0:WV0 + C], P, C,
                             f"wvf{cb}") for cb in range(2)]
            wp_f = [load_f32(wpack_ap[rows(cb), WP0:WP0 + C], P, C,
                             f"wpf{cb}") for cb in range(2)]
            bk_sb = [load_f32(wpack_ap[rows(cb), BK0:BK0 + 1], P, 1, f"bk{cb}")
                     for cb in range(2)]
            bv_sb = [load_f32(wpack_ap[rows(cb), BV0:BV0 + 1], P, 1, f"bv{cb}")
                     for cb in range(2)]
            bp_sb = [load_f32(wpack_ap[rows(cb), BP0:BP0 + 1], P, 1, f"bp{cb}")
                     for cb in range(2)]
            gnw_sb = [load_f32(wpack_ap[rows(cb), GW0:GW0 + 1], P, 1,
                               f"gnw{cb}") for cb in range(2)]
            gnb_sb = [load_f32(wpack_ap[rows(cb), GB0:GB0 + 1], P, 1,
                               f"gnb{cb}") for cb in range(2)]
            ind_sb = [load_f32(wpack_ap[rows(cb), IND0:IND0 + GROUPS], P,
                               GROUPS, f"ind{cb}") for cb in range(2)]
            indt_sb = [load_f32(
                wpack_ap[0:GROUPS, INDT0 + cb * P:INDT0 + (cb + 1) * P],
                GROUPS, P, f"indt{cb}") for cb in range(2)]

            ones_row_bf = persist.tile([1, P], bf16, tag="ones_bf",
                                       name="ones_bf")
            nc.vector.memset(ones_row_bf[:], 1.0)
            # [P, 2, 16] so the DoubleRow pair-dim step (16) satisfies the
            # fp8 dual-row LDWEIGHTS step%16==0 restriction; only col 0 used
            ones_pair_f8 = persist.tile([P, 2, 16], f8, tag="ones_pair",
                                        name="ones_pair")
            nc.vector.memset(ones_pair_f8[:], 1.0)
            eps_sb = persist.tile([GROUPS, 1], f32, tag="eps", name="eps")
            nc.vector.memset(eps_sb[:], EPS)
            neg4_sb = persist.tile([P, 1], f32, tag="neg4", name="neg4")
            nc.vector.memset(neg4_sb[:], -4.0)
            warm_w = persist.tile([P, P], bf16, tag="warmw", name="warmw")
            nc.gpsimd.memset(warm_w[:], 1.0)

            # wp doesn't depend on the GN stats; convert it early
            wpt_bf = []
            for cb in range(2):
                t = persist.tile([P, C], bf16, tag=f"wpt{cb}", name=f"wpt{cb}")
                if cb == 0:
                    nc.vector.tensor_copy(t[:], wp_f[cb][:])
                else:
                    nc.scalar.activation(t[:], wp_f[cb][:], AF.Identity)
                wpt_bf.append(t)

            DR = mybir.MatmulPerfMode.DoubleRow

            # persistent fp8 tensors
            k_f8 = persist.tile([P, 2, N], f8, tag="kf8", name="kf8")
            v_f8 = [persist.tile([P, N], f8, tag=f"vf8{cb}", name=f"vf8{cb}")
                    for cb in range(2)]
            v_att = persist.tile([P, NB // 2, 2, C], f8, tag="vatt",
                                 name="vatt")
            qT = persist.tile([P, 2, HALF], f8, tag="qT", name="qT")
            qTv = qT.rearrange("p b (a r) -> p b a r", r=16)

            # ---------- pre-attention PSUM: two [P,4,512] quads ----------
            with tc.tile_pool(name="ps4", bufs=2, space="PSUM") as ps4:
                def quad():
                    return ps4.tile([P, 4, CHUNK], f32, tag="q4", name="q4")

                # keep the PE busy through the x-DMA/GN phase: dummy matmuls
                # consuming each x chunk as it arrives hold the HAM
                # clock-gate at full rate so the convs start warm
                def junk(cb, ch):
                    jq = quad()
                    nc.tensor.matmul(
                        jq[:, 0, :], warm_w[:],
                        x_sb[cb][:, ch * CHUNK:(ch + 1) * CHUNK],
                        start=True, stop=True)

                for rep in range(2):
                    for cb in range(2):
                        for qq in range(4):
                            junk(cb, 2 * qq + rep)

                # ---------- GroupNorm stats ----------
                m1m2 = []
                for cb in range(2):
                    xv = x_sb[cb].rearrange("p (s q) -> p s q", q=512)
                    stats = small.tile([P, 8, 6], f32, tag="bnstats",
                                       name="bnstats")
                    for s in range(8):
                        nc.vector.bn_stats(stats[:, s, :], xv[:, s, :])
                    mv = small.tile([P, 2], f32, tag="bnmv", name="bnmv")
                    nc.vector.bn_aggr(mv[:], stats[:])
                    mm12 = small.tile([P, 2], f32, tag="m1m2", name="m1m2")
                    nc.vector.tensor_copy(mm12[:, 0:1], mv[:, 0:1])
                    sq = small.tile([P, 1], f32, tag="gnsq", name="gnsq")
                    nc.vector.tensor_mul(sq[:], mv[:, 0:1], mv[:, 0:1])
                    nc.vector.tensor_add(mm12[:, 1:2], mv[:, 1:2], sq[:])
                    m1m2.append(mm12)

                gq = quad()
                # group sums: [32, 2] = sum over channels in group
                nc.tensor.matmul(gq[0:GROUPS, 0, 0:2], ind_sb[0][:],
                                 m1m2[0][:], start=True, stop=False)
                nc.tensor.matmul(gq[0:GROUPS, 0, 0:2], ind_sb[1][:],
                                 m1m2[1][:], start=False, stop=True)

                gstats = small.tile([GROUPS, 2], f32, tag="gstats",
                                    name="gstats")
                nc.vector.tensor_scalar_mul(gstats[:, 0:1],
                                            gq[0:GROUPS, 0, 0:2][:, 0:1],
                                            1.0 / GSIZE)
                ex2 = small.tile([GROUPS, 1], f32, tag="gex2", name="gex2")
                nc.vector.tensor_scalar_mul(ex2[:], gq[0:GROUPS, 0, 1:2],
                                            1.0 / GSIZE)
                musq = small.tile([GROUPS, 1], f32, tag="gmusq", name="gmusq")
                nc.vector.tensor_mul(musq[:], gstats[:, 0:1], gstats[:, 0:1])
                gvar = small.tile([GROUPS, 1], f32, tag="gvar", name="gvar")
                nc.vector.tensor_sub(gvar[:], ex2[:], musq[:])
                gsd = small.tile([GROUPS, 1], f32, tag="gsd", name="gsd")
                nc.scalar.activation(gsd[:], gvar[:], AF.Sqrt, bias=eps_sb[:])
                nc.vector.reciprocal(gstats[:, 1:2], gsd[:])

                junk(0, 1)

                # scatter group stats to channels; per-channel fold params
                scale_c, bias2_bf = [], []
                sq2 = quad()
                for cb in range(2):
                    nc.tensor.matmul(sq2[:, cb, 0:2], indt_sb[cb][:],
                                     gstats[:], start=True, stop=True)
                    sc_ = small.tile([P, 1], f32, tag="scalec", name="scalec")
                    nc.vector.tensor_mul(sc_[:], sq2[:, cb, 1:2],
                                         gnw_sb[cb][:])
                    mus = small.tile([P, 1], f32, tag="mus", name="mus")
                    nc.vector.tensor_mul(mus[:], sq2[:, cb, 0:1], sc_[:])
                    bias_c = small.tile([P, 1], f32, tag="biasc", name="biasc")
                    nc.vector.tensor_sub(bias_c[:], gnb_sb[cb][:], mus[:])
                    # bias2 = bias_c / scale_c  (so W_s @ bias2 = W @ bias_c)
                    rsc = small.tile([P, 1], f32, tag="rsc", name="rsc")
                    nc.vector.reciprocal(rsc[:], sc_[:])
                    b2 = small.tile([P, 1], f32, tag="b2", name="b2")
                    nc.vector.tensor_mul(b2[:], bias_c[:], rsc[:])
                    b2b = small.tile([P, 1], bf16, tag="b2b", name="b2b")
                    nc.vector.tensor_copy(b2b[:], b2[:])
                    scale_c.append(sc_)
                    bias2_bf.append(b2b)

                junk(1, 1)

                # folded bf16 weights: W_s = W.T * scale_c (per partition)
                def fold(fsrc, cols, tag, cb, eng):
                    t = persist.tile([P, cols], bf16, tag=tag, name=tag)
                    if eng == "dve":
                        nc.vector.tensor_scalar_mul(t[:], fsrc[:],
                                                    scale_c[cb][:])
                    else:
                        nc.scalar.activation(t[:], fsrc[:], AF.Identity,
                                             scale=scale_c[cb][:])
                    return t

                wqt_s = [fold(wq_f[cb], P, f"wqs{cb}", cb,
                              "dve" if cb == 0 else "act")
                         for cb in range(2)]
                wkt_s = [fold(wk_f[cb], C, f"wks{cb}", cb,
                              "dve" if cb == 0 else "act")
                         for cb in range(2)]
                wvt_s = [fold(wv_f[cb], C, f"wvs{cb}", cb,
                              "dve" if cb == 0 else "act")
                         for cb in range(2)]

                junk(0, 3)

                # bias folds: b' = b + W @ bias_c = b + W_s @ bias2
                bq2 = quad()
                for cob in range(2):
                    for s, wt in ((0, wkt_s), (1, wvt_s)):
                        psl = bq2[:, 2 * s + cob, 0:1]
                        nc.tensor.matmul(psl, wt[0][:, cob * P:(cob + 1) * P],
                                         bias2_bf[0][:], start=True,
                                         stop=False)
                        nc.tensor.matmul(psl, wt[1][:, cob * P:(cob + 1) * P],
                                         bias2_bf[1][:], start=False,
                                         stop=True)
                bq3 = quad()
                nc.tensor.matmul(bq3[0:1, 0, 0:P], bias2_bf[0][:], wqt_s[0][:],
                                 start=True, stop=False)
                nc.tensor.matmul(bq3[0:1, 0, 0:P], bias2_bf[1][:], wqt_s[1][:],
                                 start=False, stop=True)

                bkp, bvp = [], []
                for cob in range(2):
                    t = small.tile([P, 1], f32, tag="bkp", name="bkp")
                    nc.vector.tensor_add(t[:], bq2[:, cob, 0:1],
                                         bk_sb[cob][:])
                    bkp.append(t)
                    t = small.tile([P, 1], f32, tag="bvp", name="bvp")
                    nc.vector.tensor_add(t[:], bq2[:, 2 + cob, 0:1],
                                         bv_sb[cob][:])
                    bvp.append(t)
                bq_row = small.tile([1, P], f32, tag="bqp", name="bqp")
                nc.vector.tensor_add(bq_row[:], bq3[0:1, 0, 0:P],
                                     bq_row_f[:])
                bq_bc = persist.tile([P, P], f32, tag="bqbc", name="bqbc")
                nc.gpsimd.partition_broadcast(bq_bc[:], bq_row[:])

                junk(1, 3)

                # ---------- k, v convs (K=256 via 2 bf16 matmuls) --------
                def conv_full(wt, b_sb, dst):
                    for cob in range(2):
                        for qd in range(2):
                            ps = quad()
                            for s in range(4):
                                ch = 4 * qd + s
                                sl = slice(ch * CHUNK, (ch + 1) * CHUNK)
                                nc.tensor.matmul(
                                    ps[:, s, :],
                                    wt[0][:, cob * P:(cob + 1) * P],
                                    x_sb[0][:, sl], start=True, stop=False)
                                nc.tensor.matmul(
                                    ps[:, s, :],
                                    wt[1][:, cob * P:(cob + 1) * P],
                                    x_sb[1][:, sl], start=False, stop=True)
                            for s in range(4):
                                ch = 4 * qd + s
                                sl = slice(ch * CHUNK, (ch + 1) * CHUNK)
                                if s % 2 == 0:
                                    nc.vector.tensor_scalar_add(
                                        dst(cob, sl), ps[:, s, :],
                                        b_sb[cob][:])
                                else:
                                    nc.scalar.activation(
                                        dst(cob, sl), ps[:, s, :],
                                        AF.Identity, bias=b_sb[cob][:])

                conv_full(wkt_s, bkp, lambda cob, sl: k_f8[:, cob, sl])
                conv_full(wvt_s, bvp, lambda cob, sl: v_f8[cob][:, sl])

                # v_att[j, c] = v[j//16, 256*(j%16)+c]; [j', pair, jlo, c]
                # so a [128, 2, 128] DoubleRow stationary covers two
                # j-blocks. Spread over three DMA queues.
                for jb in range(NB):
                    cb = jb // 16
                    p0 = (jb % 16) * 8
                    src = v_f8[cb][p0:p0 + 8, :].rearrange(
                        "p (r c) -> p r c", c=C)
                    eng = (nc.sync, nc.scalar, nc.gpsimd)[jb % 3]
                    eng.dma_start(v_att[:, jb // 2, jb % 2, :], src)

                # ---------- qT: q_att.T for this core's half -------------
                # qT[m, cb, 16a+r] = qconv[a, 256r+128cb+m]; bias bq'[a] is
                # added during the copies via the partition-broadcast row.
                it = [(r, cb) for r in range(16) for cb in range(2)]
                for qd in range(2):
                    ps = quad()
                    for k16 in range(16):
                        r, cb = it[qd * 16 + k16]
                        sl = slice(256 * r + cb * P, 256 * r + (cb + 1) * P)
                        psl = ps[:, k16 // 4, (k16 % 4) * P:(k16 % 4 + 1) * P]
                        nc.tensor.matmul(psl, x_sb[0][:, sl], wqt_s[0][:],
                                         start=True, stop=False)
                        nc.tensor.matmul(psl, x_sb[1][:, sl], wqt_s[1][:],
                                         start=False, stop=True)
                        nc.vector.scalar_tensor_tensor(
                            qTv[:, cb, :, r], psl, 1.0, bq_bc[:],
                            op0=OP.mult, op1=OP.add)

            # ---------- attention + projection ----------
            with (
                tc.tile_pool(name="squad", bufs=1, space="PSUM") as squad,
                tc.tile_pool(name="hacc", bufs=2, space="PSUM") as hacc,
                tc.tile_pool(name="lacc", bufs=1, space="PSUM") as lacc,
                tc.tile_pool(name="misc", bufs=1, space="PSUM") as miscp,
            ):
                def make_stage5a(hps, lp):
                    """Drain the accumulators right at chunk end so their
                    PSUM banks recycle before the next chunk's accums."""
                    l_sb = small.tile([1, CHUNK], f32, tag="lsb", name="lsb")
                    nc.vector.tensor_copy(l_sb[:], lp[:])
                    rl = small.tile([1, CHUNK], f32, tag="rl", name="rl")
                    nc.vector.reciprocal_approx_fast(rl[:], l_sb[:])
                    rl_bc = opool.tile([P, CHUNK], f32, tag="rlbc",
                                       name="rlbc")
                    nc.gpsimd.partition_broadcast(rl_bc[:], rl[:])
                    hT = [htpool.tile([P, CHUNK], bf16, tag="hT", name="hT")
                          for _ in range(2)]
                    nc.vector.tensor_copy(hT[0][:], hps[0][:])
                    nc.vector.tensor_copy(hT[1][:], hps[1][:])
                    return hT, rl_bc

                def make_stage5b(isl, hT, rl_bc):
                    def stage5b():
                        for cob in range(2):
                            pp = miscp.tile([P, CHUNK], f32, tag="pp",
                                            name="pp")
                            nc.tensor.matmul(
                                pp[:], wpt_bf[0][:, cob * P:(cob + 1) * P],
                                hT[0][:], start=True, stop=False)
                            nc.tensor.matmul(
                                pp[:], wpt_bf[1][:, cob * P:(cob + 1) * P],
                                hT[1][:], start=False, stop=True)
                            t1 = opool.tile([P, CHUNK], f32, tag="t1",
                                            name="t1")
                            nc.vector.tensor_mul(t1[:], pp[:], rl_bc[:])
                            o_t = opool.tile([P, CHUNK], bf16, tag="ot",
                                             name="ot")
                            nc.vector.tensor_scalar_add(o_t[:], t1[:],
                                                        bp_sb[cob][:])
                            nc.sync.dma_start(
                                out_ap[cob * P:(cob + 1) * P, isl], o_t[:])
                    return stage5b

                pending5 = None
                for sc in range(SC):
                    isl = slice(sc * CHUNK, (sc + 1) * CHUNK)
                    hps = None
                    lp = None

                    def accums(g, e_q):
                        """h_attT and softmax-denominator accumulation for
                        the two j-block pairs of group g (DoubleRow)."""
                        st, sp = (g == 0), (g == NG - 1)
                        for t in range(2):          # pair within group
                            pair = 2 * g + t
                            ev = e_q[:, 2 * t:2 * t + 2, :]
                            nc.tensor.matmul(hps[0][:],
                                             v_att[:, pair, :, 0:P], ev,
                                             start=st and t == 0,
                                             stop=sp and t == 1,
                                             perf_mode=DR)
                            nc.tensor.matmul(hps[1][:],
                                             v_att[:, pair, :, P:C], ev,
                                             start=st and t == 0,
                                             stop=sp and t == 1,
                                             perf_mode=DR)
                            nc.tensor.matmul(lp[:],
                                             ones_pair_f8[:, :, 0:1], ev,
                                             start=st and t == 0,
                                             stop=sp and t == 1,
                                             perf_mode=DR)

                    prev = None
                    for g in range(NG):
                        ps_s = squad.tile([P, 4, CHUNK], f32, tag="sq",
                                          name="sq")
                        for s in range(4):
                            jb = 4 * g + s
                            jsl = slice(jb * P, (jb + 1) * P)
                            nc.tensor.matmul(ps_s[:, s, :], k_f8[:, :, jsl],
                                             qT[:, :, isl], start=True,
                                             stop=True, perf_mode=DR)
                        e_q = epool.tile([P, 4, CHUNK], f8, tag="e", name="e")
                        # e^{s/16 - 4}: the -4 keeps E in fp8's finite range
                        # and cancels exactly in the softmax normalization
                        nc.scalar.activation(e_q[:], ps_s[:], AF.Exp,
                                             scale=float(C) ** -0.5,
                                             bias=neg4_sb[:])
                        if g == 1 and pending5 is not None:
                            pending5()
                            pending5 = None
                        if prev is not None:
                            if hps is None:
                                hps = [hacc.tile([P, CHUNK], f32, tag="hacc",
                                                 name="hacc")
                                       for _ in range(2)]
                                lp = lacc.tile([1, CHUNK], f32, tag="lacc",
                                               name="lacc")
                            accums(*prev)
                        prev = (g, e_q)
                    accums(*prev)
                    hT, rl_bc = make_stage5a(hps, lp)
                    pending5 = make_stage5b(isl, hT, rl_bc)
                pending5()

    nc.compile()
    return nc


def _get_exec():
    if "fn" in _cache:
        return _cache["fn"], _cache["zfn"], _cache["in_names"]

    import jax
    import jax.numpy as jnp
    import ml_dtypes
    from jax.experimental.shard_map import shard_map
    from jax.sharding import Mesh, NamedSharding, PartitionSpec as PS

    from concourse import bass2jax, mybir

    try:
        jax.config.update("jax_compilation_cache_dir", "/tmp/jax_cc_cache")
        jax.config.update("jax_persistent_cache_min_compile_time_secs", 0.0)
    except Exception:
        pass

    nc = _build_nc()
    _cache["nc"] = nc
    bass2jax.install_neuronx_cc_hook()

    partition_name = (nc.partition_id_tensor.name
                      if nc.partition_id_tensor else None)
    in_names, out_names, out_avals = [], [], []
    for alloc in nc.m.functions[0].allocations:
        if not isinstance(alloc, mybir.MemoryLocationSet):
            continue
        name = alloc.memorylocations[0].name
        if alloc.kind == "ExternalInput":
            if name != partition_name:
                in_names.append(name)
        elif alloc.kind == "ExternalOutput":
            out_avals.append(jax.core.ShapedArray(
                tuple(alloc.tensor_shape), mybir.dt.np(alloc.dtype)))
            out_names.append(name)
    n_params = len(in_names)
    all_in_names = in_names + out_names
    if partition_name:
        all_in_names = all_in_names + [partition_name]

    def _body(*args):
        operands = list(args)
        if partition_name:
            operands.append(bass2jax.partition_id_tensor())
        outs = bass2jax._bass_exec_p.bind(
            *operands, out_avals=tuple(out_avals),
            in_names=tuple(all_in_names), out_names=tuple(out_names),
            lowering_input_output_aliases=(), sim_require_finite=True,
            sim_require_nnan=True, nc=nc)
        return tuple(outs)

    devices = np.asarray(jax.devices()[:NCORES]).reshape(B, 2)
    mesh = Mesh(devices, ("pair", "half"))
    spec_by_name = {"x": PS("pair"), "qpack": PS("half"), "wpack": PS()}
    in_specs = tuple(spec_by_name[n] for n in in_names) + (PS(("pair", "half")),)
    out_specs = (PS(("pair", "half")),)

    fn = jax.jit(
        shard_map(_body, mesh=mesh, in_specs=in_specs,
                  out_specs=out_specs, check_rep=False),
        donate_argnums=(n_params,), keep_unused=True)

    zsharding = NamedSharding(mesh, PS(("pair", "half")))
    zfn = jax.jit(
        lambda: jnp.zeros((NCORES * C, HALF), ml_dtypes.bfloat16),
        out_shardings=zsharding)

    _cache["fn"] = fn
    _cache["zfn"] = zfn
    _cache["in_names"] = in_names
    return fn, zfn, in_names


def _pack_inputs(x, gn_w, gn_b, wq, bq, wk, bk, wv, bv, wp, bp):
    import ml_dtypes
    f = np.float32
    asrt = lambda a: np.asarray(a, f)
    x = np.asarray(x, f).reshape(B * C, N)
    x_bf = x.astype(ml_dtypes.bfloat16)

    wq, wk, wv, wp = asrt(wq), asrt(wk), asrt(wv), asrt(wp)
    bq, bk, bv, bp = asrt(bq), asrt(bk), asrt(bv), asrt(bp)
    gn_w, gn_b = asrt(gn_w), asrt(gn_b)

    qpack = np.empty((2 * (C + 1), P), f)
    wqT = wq.T
    for j in range(2):
        qpack[j * (C + 1):j * (C + 1) + C] = wqT[:, j * P:(j + 1) * P]
        qpack[j * (C + 1) + C] = bq[j * P:(j + 1) * P]

    wpack = np.zeros((C, WCOLS), f)
    wpack[:, WK0:WK0 + C] = wk.T
    wpack[:, WV0:WV0 + C] = wv.T
    wpack[:, WP0:WP0 + C] = wp.T
    wpack[:, BK0] = bk
    wpack[:, BV0] = bv
    wpack[:, BP0] = bp
    wpack[:, GW0] = gn_w
    wpack[:, GB0] = gn_b
    ind = np.zeros((C, GROUPS), f)
    ind[np.arange(C), np.arange(C) // GSIZE] = 1.0
    wpack[:, IND0:IND0 + GROUPS] = ind
    wpack[0:GROUPS, INDT0:INDT0 + C] = ind.T
    return x, x_bf, qpack, wpack


def kernel(x, gn_w, gn_b, wq, bq, wk, bk, wv, bv, wp, bp):
    fn, zfn, in_names = _get_exec()
    x_f32, x_bf, qpack, wpack = _pack_inputs(
        x, gn_w, gn_b, wq, bq, wk, bk, wv, bv, wp, bp)
    arrs = {"x": x_bf, "qpack": qpack, "wpack": wpack}
    (p_out,) = fn(*(arrs[n] for n in in_names), zfn())
    # p_out: (8*C, HALF) bf16, blocks ordered core = 2b + j
    p = np.asarray(p_out).astype(np.float32).reshape(B, 2, C, HALF)
    out = np.empty((B, C, N), np.float32)
    for j in range(2):
        out[:, :, j * HALF:(j + 1) * HALF] = p[:, j]
    out += x_f32.reshape(B, C, N)
    return out.reshape(B, C, H, W)
